# revision 14
# baseline (speedup 1.0000x reference)
"""Multi-head causal attention (B=2, T=4096, C=768, H=12) on 8 TRN2 NeuronCores.

Sharding: 24 (batch, head) units -> 3 heads per core; cores 0-3 take batch 0,
cores 4-7 batch 1. Each core computes Q/K/V projections for its 3 heads, full-T
causal attention, and a partial output projection [C, T] in bf16. Host sums the
4 partials per batch and adds the combined bias (bo + bv @ Wo.T).

v3 design (vs v1 baseline):
  - QK^T runs as fp8e4 DoubleRow matmuls (0.5 cyc/row, 2x bf16 throughput):
    Q/K are quantized to fp8 at natural scale (the 1/sqrt(D) lives in the Exp
    scale operand) and packed [32, 2, 512] per head with the 64 head dims
    split across (partition 32, free 2). The projection weight columns are
    pre-ordered so PSUM rows 0-63 land directly (DVE bias-add) and rows
    64-127 are partition-shifted with one small SBUF->SBUF DMA.
    Everything else stays bf16: fp8 anywhere else (V, exp, y, Wo) costs
    1.5-40% final error vs the 2% budget; fp8 Q/K alone adds ~1%.
  - Causal mask is additive on the QK PSUM before Exp (one DVE add on the
    diagonal k-pairs only).
  - Projections for tile ti+1 are interleaved into the attention k-loop of
    q-tile ti so the Scalar engine - the softmax-Exp wall at ~254us - never
    waits on a phase boundary.
  - bv never reaches the device: sum_h bv_h @ Wo_h^T is a constant [C]
    vector, added on the host with bo.
  - Partial out-projection is written bf16 (halves output DMA traffic).

Device layouts (per core):
  xT   [768, T] bf16     x[b] transposed (c-major)
  Qd,Kd [32|64, 2, 512] fp8  per-head-packed: partition = 32*h + (d%32),
                         free dim1 = d//32 (DoubleRow contraction halves)
  V    [128, 3, 65] bf16 per 128-key subtile (65th col = ones -> denominator)
  att^T [128, 2, 512] psum  QK^T transposed per (head, k-pair of subtiles)
  e    [128, 2, 512] bf16   Act Exp with scale=1/8
  y    [65, 512] psum    accum over k-subtiles (row 64 = softmax denominator)
  out  [768, T] bf16     partial out-projection, c_out-major
"""

import ml_dtypes
import numpy as np

import concourse.bass as bass
import concourse.tile as tile
from concourse import bacc, mybir

F32 = mybir.dt.float32
BF16 = mybir.dt.bfloat16
FP8 = mybir.dt.float8e4
AF = mybir.ActivationFunctionType
DR = mybir.MatmulPerfMode.DoubleRow

N_CORES = 8
T = 4096
C = 768
H = 12
D = 64
HPC = 3          # heads per core
QT = 512         # q-tile width (matmul N)
NCH = C // 128   # 6 contraction chunks over C


def build_nc(t=T):
    nt = t // QT          # q/t tiles of 512
    nsub = t // 128       # k sub-tiles of 128

    nc = bacc.Bacc("TRN2", target_bir_lowering=False, debug=False)

    xT = nc.declare_dram_parameter("xT", [C, t], BF16, isOutput=False)
    wqk = nc.declare_dram_parameter("wqk", [C, 384], BF16, isOutput=False)
    bqk = nc.declare_dram_parameter("bqk", [128, 3], F32, isOutput=False)
    wv = nc.declare_dram_parameter("wv", [C, 192], BF16, isOutput=False)
    wo = nc.declare_dram_parameter("wo", [64, 3 * C], BF16, isOutput=False)
    msk = nc.declare_dram_parameter("msk", [128, 4 * QT], BF16, isOutput=False)
    out = nc.declare_dram_parameter("out", [C, t], BF16, isOutput=True)

    xT_r = xT.ap().rearrange("(a p) t -> p a t", p=128)
    wqk_r = wqk.ap().rearrange("(a p) m -> p a m", p=128)
    wv_r = wv.ap().rearrange("(a p) m -> p a m", p=128)

    with tile.TileContext(nc) as tc:
        with (
            tc.tile_pool(name="const", bufs=1) as const_pool,
            tc.tile_pool(name="xt", bufs=2) as xp,
            tc.tile_pool(name="qd", bufs=nt) as qdp,
            tc.tile_pool(name="kd", bufs=nt) as kdp,
            tc.tile_pool(name="q2d", bufs=nt) as q2dp,
            tc.tile_pool(name="k2d", bufs=nt) as k2dp,
            tc.tile_pool(name="vp", bufs=nsub) as vp,
            tc.tile_pool(name="ep", bufs=6) as ep,
            tc.tile_pool(name="yp", bufs=HPC + 1) as yp,
            tc.tile_pool(name="sp", bufs=8) as sp,
            tc.tile_pool(name="tq", bufs=3) as tqp,
            tc.tile_pool(name="op", bufs=4) as op,
            tc.tile_pool(name="ps_qk", bufs=2, space="PSUM") as ps_qk,
            tc.tile_pool(name="ps_y", bufs=HPC, space="PSUM") as ps_y,
            tc.tile_pool(name="ps_pr", bufs=1, space="PSUM") as ps_pr,
        ):
            # ---- constants ----
            wqk_sb = const_pool.tile([128, NCH, 384], BF16, tag="wqk")
            nc.sync.dma_start(out=wqk_sb, in_=wqk_r)
            wv_sb = const_pool.tile([128, NCH, 192], BF16, tag="wv")
            nc.sync.dma_start(out=wv_sb, in_=wv_r)
            wo_sb = const_pool.tile([64, 3 * C], BF16, tag="wo")
            nc.sync.dma_start(out=wo_sb, in_=wo.ap())
            bqk_sb = const_pool.tile([128, 3], F32, tag="bqk")
            nc.sync.dma_start(out=bqk_sb, in_=bqk.ap())
            mask_sb = const_pool.tile([128, 4, QT], BF16, tag="msk")
            msk_r = msk.ap().rearrange("p (o q) -> p o q", q=QT)
            nc.sync.dma_start(out=mask_sb, in_=msk_r)

            q_d = [None] * nt   # [64, 2, 512] fp8: heads 0,1 packed
            k_d = [None] * nt
            q2_d = [None] * nt  # [32, 2, 512] fp8: head 2
            k2_d = [None] * nt
            v_t = [None] * nsub
            x_t = [None] * nt
            y_t = [[None] * nt for _ in range(HPC)]

            def fetch_x(ti):
                xt = xp.tile([128, NCH, QT], BF16, tag="xt")
                nc.sync.dma_start(out=xt, in_=xT_r[:, :, ti * QT:(ti + 1) * QT])
                x_t[ti] = xt

            def emit_proj_piece(ti, piece):
                """pieces 0-2: Q01/K01/Q2K2 m-tiles (fp8-packed); 3-6: V."""
                xt = x_t[ti]
                if piece < 3:
                    ps = ps_pr.tile([128, QT], F32, tag="pp")
                    for ci in range(NCH):
                        nc.tensor.matmul(
                            ps,
                            lhsT=wqk_sb[:, ci, piece * 128:(piece + 1) * 128],
                            rhs=xt[:, ci, :],
                            start=(ci == 0),
                            stop=(ci == NCH - 1),
                        )
                    bias = bqk_sb[:, piece:piece + 1]
                    if piece < 2:
                        # psum rows: 0-63 = (j=0, h0 d0-31|h1 d0-31),
                        #           64-127 = (j=1, same) -> DMA shift
                        dev = (qdp, kdp)[piece].tile(
                            [64, 2, QT], FP8, tag=("qd", "kd")[piece])
                        nc.vector.tensor_scalar_add(
                            dev[:, 0, :], ps[0:64, :], bias[0:64])
                        tmp = tqp.tile([128, QT], FP8, tag="tq")
                        nc.vector.tensor_scalar_add(
                            tmp[64:128, :], ps[64:128, :], bias[64:128])
                        nc.sync.dma_start(out=dev[:, 1, :], in_=tmp[64:128, :])
                        (q_d, k_d)[piece][ti] = dev
                    else:
                        # rows 0-31 q2 j0 | 32-63 q2 j1 | 64-95 k2 j0 |
                        # 96-127 k2 j1
                        q2 = q2dp.tile([32, 2, QT], FP8, tag="q2d")
                        k2 = k2dp.tile([32, 2, QT], FP8, tag="k2d")
                        nc.vector.tensor_scalar_add(
                            q2[:, 0, :], ps[0:32, :], bias[0:32])
                        tmp = tqp.tile([128, QT], FP8, tag="tq")
                        nc.vector.tensor_scalar_add(
                            tmp[32:64, :], ps[32:64, :], bias[32:64])
                        nc.vector.tensor_scalar_add(
                            tmp[64:128, :], ps[64:128, :], bias[64:128])
                        nc.sync.dma_start(out=q2[:, 1, :], in_=tmp[32:64, :])
                        nc.sync.dma_start(out=k2[:, 0, :], in_=tmp[64:96, :])
                        nc.sync.dma_start(out=k2[:, 1, :], in_=tmp[96:128, :])
                        q2_d[ti] = q2
                        k2_d[ti] = k2
                else:
                    si = piece - 3
                    ps = ps_pr.tile([128, QT], F32, tag="pp")
                    for ci in range(NCH):
                        nc.tensor.matmul(
                            ps[:, 0:HPC * 64],
                            lhsT=xt[:, ci, si * 128:(si + 1) * 128],
                            rhs=wv_sb[:, ci, :],
                            start=(ci == 0),
                            stop=(ci == NCH - 1),
                        )
                    vt = vp.tile([128, HPC, 65], BF16, tag="v")
                    nc.vector.memset(vt[:, :, 64:65], 1.0)
                    nc.vector.tensor_copy(
                        vt[:, :, 0:64],
                        ps[:, 0:HPC * 64].rearrange("p (h e) -> p h e", e=64),
                    )
                    v_t[ti * 4 + si] = vt

            def normalize(h, qi, y_ps):
                # y_ps row 64 = sum(e). GpSimd cannot read PSUM: DVE-copy the
                # row to SBUF, DMA it to partition 0, broadcast, reciprocal,
                # then scale rows 0-63.
                den64 = sp.tile([65, QT], F32, tag="den64")
                nc.vector.tensor_copy(den64[64:65, :], y_ps[64:65, :])
                den = sp.tile([1, QT], F32, tag="den")
                nc.sync.dma_start(out=den, in_=den64[64:65, :])
                bc = sp.tile([64, QT], F32, tag="bc")
                nc.gpsimd.partition_broadcast(bc, den[0:1, :])
                rec = sp.tile([64, QT], F32, tag="rec")
                nc.vector.reciprocal_approx_fast(rec, bc)
                yt = yp.tile([64, QT], BF16, tag="y")
                nc.vector.tensor_mul(yt, y_ps[0:64, :], rec)
                y_t[h][qi] = yt

            # ---- pipeline: proj(0), then per qi attention with proj(qi+1)
            # pieces interleaved into the k-loop ----
            fetch_x(0)
            for piece in range(7):
                emit_proj_piece(0, piece)

            for qi in range(nt):
                n_kp = 2 * (qi + 1)
                if qi + 1 < nt:
                    fetch_x(qi + 1)
                pending = list(range(7)) if qi + 1 < nt else []

                y0 = ps_y.tile([65, QT], F32, tag="psy")
                y1 = ps_y.tile([65, QT], F32, tag="psy")
                y2 = ps_y.tile([65, QT], F32, tag="psy")
                ys = [y0, y1, y2]

                for kp in range(n_kp):
                    o = 2 * kp - 4 * qi  # diag-block subtile offset (u=0)
                    for h in range(HPC):
                        if h < 2:
                            qdev = q_d[qi][32 * h:32 * h + 32, :, :]
                            kd_f = k_d
                        else:
                            qdev = q2_d[qi][:, :, :]
                            kd_f = k2_d
                        aps = ps_qk.tile([128, 2, QT], F32, tag="qk")
                        for u in (0, 1):
                            s = 2 * kp + u
                            tj, tcol = s // 4, (s % 4) * 128
                            kt = kd_f[tj]
                            lhsT = (kt[32 * h:32 * h + 32, :, tcol:tcol + 128]
                                    if h < 2 else kt[:, :, tcol:tcol + 128])
                            nc.tensor.matmul(
                                aps[:, u, :], lhsT=lhsT, rhs=qdev,
                                start=True, stop=True, perf_mode=DR,
                            )
                        if o >= 0:
                            nc.vector.tensor_add(aps, aps, mask_sb[:, o:o + 2, :])
                        et = ep.tile([128, 2, QT], BF16, tag="e")
                        nc.scalar.activation(et, aps, AF.Exp, scale=0.125)
                        for u in (0, 1):
                            s = 2 * kp + u
                            nc.tensor.matmul(
                                ys[h],
                                lhsT=v_t[s][:, h, :],
                                rhs=et[:, u, :],
                                start=(s == 0),
                                stop=(s == 4 * qi + 3),
                            )
                    if pending:
                        emit_proj_piece(qi + 1, pending.pop(0))
                while pending:
                    emit_proj_piece(qi + 1, pending.pop(0))

                for h in range(HPC):
                    normalize(h, qi, ys[h])

                # out-projection partial for this q-tile
                for mo in range(NCH):
                    ps = ps_pr.tile([128, QT], F32, tag="pp")
                    for h in range(HPC):
                        nc.tensor.matmul(
                            ps,
                            lhsT=wo_sb[:, h * C + mo * 128:h * C + (mo + 1) * 128],
                            rhs=y_t[h][qi],
                            start=(h == 0),
                            stop=(h == HPC - 1),
                        )
                    ot = op.tile([128, QT], BF16, tag="o")
                    nc.vector.tensor_copy(ot, ps)
                    nc.sync.dma_start(
                        out=out.ap()[mo * 128:(mo + 1) * 128,
                                     qi * QT:(qi + 1) * QT],
                        in_=ot,
                    )

    nc.compile()
    return nc


def make_mask():
    """Additive causal mask per diag-block offset o: 0 where valid, -30000."""
    i = np.arange(128)[:, None]
    j = np.arange(QT)[None, :]
    m = np.zeros((128, 4 * QT), np.float32)
    for o in range(4):
        m[:, o * QT:(o + 1) * QT] = np.where(j >= o * 128 + i, 0.0, -30000.0)
    return m


def _pack_qk_cols(Wh):
    """[64*2, C] rows (h*64 + d) -> m order j*64 + h*32 + r, d = 32j + r."""
    W = Wh.reshape(2, 2, 32, -1)      # [h, j, r, C]
    return W.transpose(1, 0, 2, 3).reshape(128, -1)


def _pack_qk_cols2(Wh):
    """[64, C] head-2 rows d -> m order j*32 + r."""
    return Wh.reshape(2, 32, -1).reshape(64, -1)


def shard_inputs(x, Wq, bq, Wk, bk, Wv, bv, Wo, bo, t=T):
    """Build per-core in_maps."""
    mask = make_mask()
    bf = ml_dtypes.bfloat16
    in_maps = []
    for c in range(N_CORES):
        b = c // (N_CORES // x.shape[0])
        h0 = HPC * (c % 4)
        hs = slice(h0 * D, (h0 + HPC) * D)
        Wq_s, bq_s = Wq[hs], bq[hs]
        Wk_s, bk_s = Wk[hs], bk[hs]
        # m-tile 0: Q01 packed; 1: K01 packed; 2: [Q2(2x32) | K2(2x32)]
        wqk = np.concatenate(
            [
                _pack_qk_cols(Wq_s[0:128]).T,
                _pack_qk_cols(Wk_s[0:128]).T,
                _pack_qk_cols2(Wq_s[128:192]).T,
                _pack_qk_cols2(Wk_s[128:192]).T,
            ],
            axis=1,
        )  # [768, 384]
        bqk = np.zeros((128, 3), np.float32)
        bqk[:, 0] = _pack_qk_cols(bq_s[0:128].reshape(128, 1))[:, 0]
        bqk[:, 1] = _pack_qk_cols(bk_s[0:128].reshape(128, 1))[:, 0]
        bqk[0:64, 2] = _pack_qk_cols2(bq_s[128:192].reshape(64, 1))[:, 0]
        bqk[64:128, 2] = _pack_qk_cols2(bk_s[128:192].reshape(64, 1))[:, 0]
        wv = np.ascontiguousarray(Wv[hs].T)
        wo = np.concatenate(
            [Wo[:, hs][:, h * D:(h + 1) * D].T for h in range(HPC)], axis=1
        )  # [64, 3*768]
        in_maps.append({
            "xT": np.ascontiguousarray(x[b].T).astype(bf),
            "wqk": np.ascontiguousarray(wqk).astype(bf),
            "bqk": np.ascontiguousarray(bqk),
            "wv": wv.astype(bf),
            "wo": np.ascontiguousarray(wo).astype(bf),
            "msk": mask.astype(bf),
        })
    return in_maps


_NC_CACHE = {}


def get_nc(t=T):
    if t not in _NC_CACHE:
        _NC_CACHE[t] = build_nc(t)
    return _NC_CACHE[t]


def run_cores(in_maps, t=T, trace=False, tmpdir=None):
    from concourse.bass_utils import run_bass_kernel_spmd

    nc = get_nc(t)
    return run_bass_kernel_spmd(
        nc, in_maps, list(range(N_CORES)), trace=trace, tmpdir=tmpdir
    )


def gather(results, x_shape, bv, Wo, bo):
    B, t, _ = x_shape
    out = np.zeros((B, t, C), np.float32)
    for c in range(N_CORES):
        b = c // (N_CORES // B)
        out[b] += results[c]["out"].T.astype(np.float32)
    out += (bv @ Wo.T + bo)[None, None, :]
    return out


def kernel(x, Wq, bq, Wk, bk, Wv, bv, Wo, bo, _trace=False, _tmpdir=None):
    x = np.asarray(x, dtype=np.float32)
    args = [np.asarray(a, dtype=np.float32) for a in (Wq, bq, Wk, bk, Wv, bv, Wo, bo)]
    Wq, bq, Wk, bk, Wv, bv, Wo, bo = args
    t = x.shape[1]
    in_maps = shard_inputs(x, Wq, bq, Wk, bk, Wv, bv, Wo, bo, t=t)
    res = run_cores(in_maps, t=t, trace=_trace, tmpdir=_tmpdir)
    out = gather(res.results, x.shape, bv, Wo, bo)
    kernel.last_result = res
    return out


# revision 19
# speedup vs baseline: 1.6889x; 1.6889x over previous
"""Multi-head causal attention (B=2, T=4096, C=768, H=12) on 8 TRN2 NeuronCores.

Sharding: 24 (batch, head) units -> 3 heads per core; cores 0-3 take batch 0,
cores 4-7 batch 1. Each core computes Q/K/V projections for its 3 heads, full-T
causal attention, and a partial output projection [C, T] in bf16. Host sums the
4 partials per batch and adds the combined bias (bo + bv @ Wo.T).

v4 (vs v1 baseline; all-bf16 matmuls - fp8/DoubleRow measured slower on hw):
  - Projections for tile ti+1 are interleaved into the attention loops of
    q-tile ti, so the Scalar engine (the softmax-Exp wall, ~255us) never
    waits at a phase boundary and the PE pipeline stays fed.
  - Causal trim: diagonal-block QK^T and att@V matmuls only stream the
    q-columns at-or-after the diagonal; the masked-out region of the exp
    tile is zeroed by the mask multiply and never re-streamed.
  - Head 2 lives at partitions 64-127 (PE rows 64-127), so its QK^T matmuls
    overlap the out-projection matmuls (rows 0-63) they are interleaved
    with; heads 0/1 already pair up as PE row groups 0-63/64-127.
  - Out-projection partials are written bf16 (halves output DMA);
    bv never reaches the device (sum_h bv_h @ Wo_h^T is a host constant).

Device layouts (per core):
  xT   [768, T] bf16   x[b] transposed (c-major)
  Q, K [d, t] bf16     head-pair tiles [128, 512] (partitions = 2x64 head
                       dims); head2 in [128, 512] tiles using rows 64-127
  V    [t, d] bf16     per 128-row tile [128, 3*65] (65th col = ones -> denom)
  att^T [k, q]         QK^T computed transposed (lhsT=K-tile, rhs=Q-tile);
                       heads 0/1 interleaved -> concurrent PE row groups
  exp   bf16           ACT Exp from PSUM, causal mask applied as 0/1 multiply
  y^T  [65, 512] psum  accum over k-tiles (row 64 = softmax denominator)
  out  [768, T] bf16   partial out-projection, c_out-major
"""

import ml_dtypes
import numpy as np

import concourse.bass as bass
import concourse.tile as tile
from concourse import bacc, mybir

F32 = mybir.dt.float32
BF16 = mybir.dt.bfloat16
AF = mybir.ActivationFunctionType

N_CORES = 8
T = 4096
C = 768
H = 12
D = 64
HPC = 3          # heads per core
QT = 512         # q-tile width (matmul N)
KT = 128         # k-tile width (partition dim)
NCH = C // 128   # 6 contraction chunks over C

# m-tile column ranges in wqk: [Qh0|Qh1](128), [Kh0|Kh1](128), [Qh2](64),
# [Kh2](64); head2 m-tiles land at PSUM rows 64-127.
M_COLS = [(0, 128), (128, 256), (256, 320), (320, 384)]


def build_nc(t=T):
    nt = t // QT          # q/t tiles of 512
    nsub = t // KT        # t sub-tiles of 128

    nc = bacc.Bacc("TRN2", target_bir_lowering=False, debug=False)

    xT = nc.declare_dram_parameter("xT", [C, t], BF16, isOutput=False)
    wqk = nc.declare_dram_parameter("wqk", [C, 384], BF16, isOutput=False)
    bqk = nc.declare_dram_parameter("bqk", [128, 4], F32, isOutput=False)
    wv = nc.declare_dram_parameter("wv", [C, 192], BF16, isOutput=False)
    wo = nc.declare_dram_parameter("wo", [64, 3 * C], BF16, isOutput=False)
    msk = nc.declare_dram_parameter("msk", [128, 4 * QT], BF16, isOutput=False)
    out = nc.declare_dram_parameter("out", [C, t], BF16, isOutput=True)

    xT_r = xT.ap().rearrange("(a p) t -> p a t", p=128)
    wqk_r = wqk.ap().rearrange("(a p) m -> p a m", p=128)
    wv_r = wv.ap().rearrange("(a p) m -> p a m", p=128)

    with tile.TileContext(nc) as tc:
        with (
            tc.tile_pool(name="const", bufs=1) as const_pool,
            tc.tile_pool(name="xt", bufs=3) as xp,
            tc.tile_pool(name="qp", bufs=nt) as qp,
            tc.tile_pool(name="kp", bufs=nt) as kpool,
            tc.tile_pool(name="q2p", bufs=nt) as q2p,
            tc.tile_pool(name="k2p", bufs=nt) as k2p,
            tc.tile_pool(name="vp", bufs=nsub) as vp,
            tc.tile_pool(name="yp", bufs=HPC + 1) as yp,
            tc.tile_pool(name="ep", bufs=8) as ep,
            tc.tile_pool(name="op", bufs=4) as op,
            tc.tile_pool(name="sp", bufs=6) as sp,
            tc.tile_pool(name="ps_qk", bufs=2, space="PSUM") as ps_qk,
            tc.tile_pool(name="ps_y", bufs=HPC, space="PSUM") as ps_y,
            tc.tile_pool(name="ps_pr", bufs=1, space="PSUM") as ps_pr,
        ):
            # ---- constants ----
            wqk_sb = const_pool.tile([128, NCH, 384], BF16, tag="wqk")
            nc.sync.dma_start(out=wqk_sb, in_=wqk_r)
            wv_sb = const_pool.tile([128, NCH, 192], BF16, tag="wv")
            nc.sync.dma_start(out=wv_sb, in_=wv_r)
            wo_sb = const_pool.tile([64, 3 * C], BF16, tag="wo")
            nc.sync.dma_start(out=wo_sb, in_=wo.ap())
            bqk_sb = const_pool.tile([128, 4], F32, tag="bqk")
            nc.sync.dma_start(out=bqk_sb, in_=bqk.ap())
            mask_sb = const_pool.tile([128, 4, QT], BF16, tag="msk")
            msk_r = msk.ap().rearrange("p (o q) -> p o q", q=QT)
            nc.sync.dma_start(out=mask_sb, in_=msk_r)

            q_t, k_t = [None] * nt, [None] * nt
            q2_t, k2_t = [None] * nt, [None] * nt   # [128,512], rows 64-127
            v_t = [None] * nsub
            x_t = [None] * nt
            y_t = [[None] * nt for _ in range(HPC)]

            def fetch_x(ti):
                xt = xp.tile([128, NCH, QT], BF16, tag="xt")
                nc.sync.dma_start(out=xt, in_=xT_r[:, :, ti * QT:(ti + 1) * QT])
                x_t[ti] = xt

            def emit_proj_piece(ti, piece):
                """pieces 0-3: Q01/K01/Q2/K2 m-tiles; 4-7: V k-subtiles."""
                xt = x_t[ti]
                if piece < 4:
                    c0, c1 = M_COLS[piece]
                    mp = c1 - c0
                    r0 = 0 if piece < 2 else 64   # head2 at rows 64-127
                    ps = ps_pr.tile([128, QT], F32, tag="pp")
                    for ci in range(NCH):
                        nc.tensor.matmul(
                            ps[r0:r0 + mp, :],
                            lhsT=wqk_sb[:, ci, c0:c1],
                            rhs=xt[:, ci, :],
                            start=(ci == 0),
                            stop=(ci == NCH - 1),
                        )
                    if piece < 2:
                        pool = (qp, kpool)[piece]
                        dst = pool.tile([128, QT], BF16, tag=("q", "k")[piece])
                        nc.vector.tensor_scalar_add(
                            dst, ps, bqk_sb[:, piece:piece + 1])
                        (q_t, k_t)[piece][ti] = dst
                    else:
                        pool = (q2p, k2p)[piece - 2]
                        full = pool.tile([128, QT], BF16, tag=("q2", "k2")[piece - 2])
                        nc.vector.tensor_scalar_add(
                            full[64:128, :], ps[64:128, :],
                            bqk_sb[64:128, piece:piece + 1])
                        (q2_t, k2_t)[piece - 2][ti] = full
                else:
                    si = piece - 4
                    ps = ps_pr.tile([128, QT], F32, tag="pp")
                    for ci in range(NCH):
                        nc.tensor.matmul(
                            ps[:, 0:HPC * 64],
                            lhsT=xt[:, ci, si * 128:(si + 1) * 128],
                            rhs=wv_sb[:, ci, :],
                            start=(ci == 0),
                            stop=(ci == NCH - 1),
                        )
                    vt = vp.tile([128, HPC * 65], BF16, tag="v")
                    vt_r = vt.rearrange("p (h e) -> p h e", e=65)
                    nc.vector.memset(vt_r[:, :, 64:65], 1.0)
                    nc.vector.tensor_copy(
                        vt_r[:, :, 0:64],
                        ps[:, 0:HPC * 64].rearrange("p (h e) -> p h e", e=64),
                    )
                    v_t[ti * 4 + si] = vt

            def normalize(h, qi, y_ps):
                # y_ps row 64 = denominator. DVE-copy to SBUF partition 64,
                # DMA to partition 0, broadcast on GpSimd, reciprocal, scale.
                den64 = sp.tile([65, QT], F32, tag="den64")
                nc.vector.tensor_copy(den64[64:65, :], y_ps[64:65, :])
                den = sp.tile([1, QT], F32, tag="den")
                nc.sync.dma_start(out=den, in_=den64[64:65, :])
                bc_sb = sp.tile([64, QT], F32, tag="bcs")
                nc.gpsimd.partition_broadcast(bc_sb, den[0:1, :])
                rec = sp.tile([64, QT], F32, tag="rec")
                nc.vector.reciprocal_approx_fast(rec, bc_sb)
                yt = yp.tile([64, QT], BF16, tag="y")
                nc.vector.tensor_mul(yt, y_ps[0:64, :], rec)
                y_t[h][qi] = yt

            def emit_outproj(qi):
                for mo in range(NCH):
                    ps = ps_pr.tile([128, QT], F32, tag="pp")
                    for h in range(HPC):
                        nc.tensor.matmul(
                            ps,
                            lhsT=wo_sb[:, h * C + mo * 128:h * C + (mo + 1) * 128],
                            rhs=y_t[h][qi],
                            start=(h == 0),
                            stop=(h == HPC - 1),
                        )
                    ot = op.tile([128, QT], BF16, tag="o")
                    nc.vector.tensor_copy(ot, ps)
                    nc.sync.dma_start(
                        out=out.ap()[mo * 128:(mo + 1) * 128,
                                     qi * QT:(qi + 1) * QT],
                        in_=ot,
                    )
                    yield

            # ---- pipeline ----
            fetch_x(0)
            for piece in range(8):
                emit_proj_piece(0, piece)

            for qi in range(nt):
                n_k = 4 * qi + 4
                if qi + 1 < nt:
                    fetch_x(qi + 1)
                pending = list(range(8)) if qi + 1 < nt else []
                # out-proj of the previous q-tile: interleave its pieces with
                # head2's QK (PE rows 64-127 vs 0-63 -> they overlap)
                oproj = emit_outproj(qi - 1) if qi > 0 else iter(())

                y0 = ps_y.tile([65, QT], F32, tag="psy")
                y1 = ps_y.tile([65, QT], F32, tag="psy")
                y2 = ps_y.tile([65, QT], F32, tag="psy")

                # head 2 (PE rows 64-127): pairs of k-tiles per Exp
                def qlo_of(kt):
                    o = kt - 4 * qi
                    return 128 * o if (o > 0 and qi > 0) else 0

                q2_ap = q2_t[qi][64:128, :]
                for pj in range(n_k // 2):
                    kts = (2 * pj, 2 * pj + 1)
                    qlo0 = qlo_of(kts[0])  # <= qlo_of(kts[1])
                    aps = ps_qk.tile([128, 2, QT], F32, tag="ps")
                    for u in (0, 1):
                        kt = kts[u]
                        tj, tcol = kt // 4, (kt % 4) * 128
                        # write from the pair's min trim so the shared Exp
                        # reads only matmul-written columns
                        nc.tensor.matmul(
                            aps[:, u, qlo0:], lhsT=k2_t[tj][64:128, tcol:tcol + 128],
                            rhs=q2_ap[:, qlo0:], start=True, stop=True,
                        )
                    et = ep.tile([128, 2, QT], BF16, tag="e")
                    nc.scalar.activation(et[:, :, qlo0:], aps[:, :, qlo0:], AF.Exp)
                    for u in (0, 1):
                        o = kts[u] - 4 * qi
                        if o >= 0:
                            qlo = qlo_of(kts[u])
                            nc.vector.tensor_mul(
                                et[:, u, qlo:], et[:, u, qlo:], mask_sb[:, o, qlo:]
                            )
                    for u in (0, 1):
                        kt = kts[u]
                        qlo = qlo_of(kt)
                        nc.tensor.matmul(
                            y2[:, qlo:],
                            lhsT=v_t[kt][:, 2 * 65:3 * 65],
                            rhs=et[:, u, qlo:],
                            start=(kt == 0),
                            stop=(kt == n_k - 1),
                        )
                    next(oproj, None)
                    next(oproj, None)
                for _ in oproj:
                    pass

                # heads 0/1 interleaved per k-tile: PE row groups 0-63/64-127
                q0_ap = q_t[qi][0:64, :]
                q1_ap = q_t[qi][64:128, :]
                for kt in range(n_k):
                    tj, tcol = kt // 4, (kt % 4) * 128
                    o = kt - 4 * qi
                    qlo = 128 * o if (o > 0 and qi > 0) else 0
                    aps = ps_qk.tile([128, 2, QT], F32, tag="ps")
                    nc.tensor.matmul(
                        aps[:, 0, qlo:], lhsT=k_t[tj][0:64, tcol:tcol + 128],
                        rhs=q0_ap[:, qlo:], start=True, stop=True,
                    )
                    nc.tensor.matmul(
                        aps[:, 1, qlo:], lhsT=k_t[tj][64:128, tcol:tcol + 128],
                        rhs=q1_ap[:, qlo:], start=True, stop=True,
                    )
                    et = ep.tile([128, 2, QT], BF16, tag="e")
                    nc.scalar.activation(et[:, :, qlo:], aps[:, :, qlo:], AF.Exp)
                    if o >= 0:
                        nc.vector.tensor_mul(
                            et[:, :, qlo:], et[:, :, qlo:],
                            mask_sb[:, o:o + 1, qlo:].to_broadcast(
                                [128, 2, QT - qlo]),
                        )
                    for u, yps in ((0, y0), (1, y1)):
                        nc.tensor.matmul(
                            yps[:, qlo:],
                            lhsT=v_t[kt][:, u * 65:(u + 1) * 65],
                            rhs=et[:, u, qlo:],
                            start=(kt == 0),
                            stop=(kt == n_k - 1),
                        )
                    if pending:
                        emit_proj_piece(qi + 1, pending.pop(0))
                while pending:
                    emit_proj_piece(qi + 1, pending.pop(0))

                normalize(0, qi, y0)
                normalize(1, qi, y1)
                normalize(2, qi, y2)

            for _ in emit_outproj(nt - 1):
                pass

    nc.compile()
    return nc


def make_mask():
    i = np.arange(128)[:, None]
    j = np.arange(QT)[None, :]
    m = np.zeros((128, 4 * QT), np.float32)
    for o in range(4):
        m[:, o * QT:(o + 1) * QT] = (j >= o * 128 + i)
    return m


def shard_inputs(x, Wq, bq, Wk, bk, Wv, bv, Wo, bo, t=T):
    """Build per-core in_maps."""
    s = 1.0 / np.sqrt(D)
    mask = make_mask()
    bf = ml_dtypes.bfloat16
    in_maps = []
    for c in range(N_CORES):
        b = c // (N_CORES // x.shape[0])
        h0 = HPC * (c % 4)
        hs = slice(h0 * D, (h0 + HPC) * D)
        Wq_s = (Wq[hs] * s).astype(np.float32)
        bq_s = (bq[hs] * s).astype(np.float32)
        Wk_s, bk_s = Wk[hs], bk[hs]
        wqk = np.concatenate(
            [Wq_s[0:128].T, Wk_s[0:128].T, Wq_s[128:192].T, Wk_s[128:192].T], axis=1
        )  # [768, 384]
        bqk = np.zeros((128, 4), np.float32)
        bqk[:, 0] = bq_s[0:128]
        bqk[:, 1] = bk_s[0:128]
        bqk[64:128, 2] = bq_s[128:192]
        bqk[64:128, 3] = bk_s[128:192]
        wv = np.ascontiguousarray(Wv[hs].T)
        wo = np.concatenate(
            [Wo[:, hs][:, h * D:(h + 1) * D].T for h in range(HPC)], axis=1
        )  # [64, 3*768]
        in_maps.append({
            "xT": np.ascontiguousarray(x[b].T).astype(bf),
            "wqk": np.ascontiguousarray(wqk).astype(bf),
            "bqk": np.ascontiguousarray(bqk),
            "wv": wv.astype(bf),
            "wo": np.ascontiguousarray(wo).astype(bf),
            "msk": mask.astype(bf),
        })
    return in_maps


_NC_CACHE = {}


def get_nc(t=T):
    if t not in _NC_CACHE:
        _NC_CACHE[t] = build_nc(t)
    return _NC_CACHE[t]


def run_cores(in_maps, t=T, trace=False, tmpdir=None):
    from concourse.bass_utils import run_bass_kernel_spmd

    nc = get_nc(t)
    return run_bass_kernel_spmd(
        nc, in_maps, list(range(N_CORES)), trace=trace, tmpdir=tmpdir
    )


def gather(results, x_shape, bv, Wo, bo):
    B, t, _ = x_shape
    out = np.zeros((B, t, C), np.float32)
    for c in range(N_CORES):
        b = c // (N_CORES // B)
        out[b] += results[c]["out"].T.astype(np.float32)
    out += (bv @ Wo.T + bo)[None, None, :]
    return out


def kernel(x, Wq, bq, Wk, bk, Wv, bv, Wo, bo, _trace=False, _tmpdir=None):
    x = np.asarray(x, dtype=np.float32)
    args = [np.asarray(a, dtype=np.float32) for a in (Wq, bq, Wk, bk, Wv, bv, Wo, bo)]
    Wq, bq, Wk, bk, Wv, bv, Wo, bo = args
    t = x.shape[1]
    in_maps = shard_inputs(x, Wq, bq, Wk, bk, Wv, bv, Wo, bo, t=t)
    res = run_cores(in_maps, t=t, trace=_trace, tmpdir=_tmpdir)
    out = gather(res.results, x.shape, bv, Wo, bo)
    kernel.last_result = res
    return out


# revision 21
# speedup vs baseline: 1.7337x; 1.0265x over previous
"""Multi-head causal attention (B=2, T=4096, C=768, H=12) on 8 TRN2 NeuronCores.

Sharding: 24 (batch, head) units -> 3 heads per core; cores 0-3 take batch 0,
cores 4-7 batch 1. Each core computes Q/K/V projections for its 3 heads, full-T
causal attention, and a partial output projection [C, T] in bf16. Host sums the
4 partials per batch and adds the combined bias (bo + bv @ Wo.T).

v4 (vs v1 baseline; all-bf16 matmuls - fp8/DoubleRow measured slower on hw):
  - Projections for tile ti+1 are interleaved into the attention loops of
    q-tile ti, so the Scalar engine (the softmax-Exp wall, ~255us) never
    waits at a phase boundary and the PE pipeline stays fed.
  - Causal trim: diagonal-block QK^T and att@V matmuls only stream the
    q-columns at-or-after the diagonal; the masked-out region of the exp
    tile is zeroed by the mask multiply and never re-streamed.
  - Head 2 lives at partitions 64-127 (PE rows 64-127), so its QK^T matmuls
    overlap the out-projection matmuls (rows 0-63) they are interleaved
    with; heads 0/1 already pair up as PE row groups 0-63/64-127.
  - Out-projection partials are written bf16 (halves output DMA);
    bv never reaches the device (sum_h bv_h @ Wo_h^T is a host constant).

Device layouts (per core):
  xT   [768, T] bf16   x[b] transposed (c-major)
  Q, K [d, t] bf16     head-pair tiles [128, 512] (partitions = 2x64 head
                       dims); head2 in [128, 512] tiles using rows 64-127
  V    [t, d] bf16     per 128-row tile [128, 3*65] (65th col = ones -> denom)
  att^T [k, q]         QK^T computed transposed (lhsT=K-tile, rhs=Q-tile);
                       heads 0/1 interleaved -> concurrent PE row groups
  exp   bf16           ACT Exp from PSUM, causal mask applied as 0/1 multiply
  y^T  [65, 512] psum  accum over k-tiles (row 64 = softmax denominator)
  out  [768, T] bf16   partial out-projection, c_out-major
"""

import ml_dtypes
import numpy as np

import concourse.bass as bass
import concourse.tile as tile
from concourse import bacc, mybir

F32 = mybir.dt.float32
BF16 = mybir.dt.bfloat16
AF = mybir.ActivationFunctionType

N_CORES = 8
T = 4096
C = 768
H = 12
D = 64
HPC = 3          # heads per core
QT = 512         # q-tile width (matmul N)
KT = 128         # k-tile width (partition dim)
NCH = C // 128   # 6 contraction chunks over C

# m-tile column ranges in wqk: [Qh0|Qh1](128), [Kh0|Kh1](128), [Qh2](64),
# [Kh2](64); head2 m-tiles land at PSUM rows 64-127.
M_COLS = [(0, 128), (128, 256), (256, 320), (320, 384)]


def build_nc(t=T):
    nt = t // QT          # q/t tiles of 512
    nsub = t // KT        # t sub-tiles of 128

    nc = bacc.Bacc("TRN2", target_bir_lowering=False, debug=False)

    xT = nc.declare_dram_parameter("xT", [C, t], BF16, isOutput=False)
    wqk = nc.declare_dram_parameter("wqk", [C, 384], BF16, isOutput=False)
    bqk = nc.declare_dram_parameter("bqk", [128, 4], F32, isOutput=False)
    wv = nc.declare_dram_parameter("wv", [C, 192], BF16, isOutput=False)
    wo = nc.declare_dram_parameter("wo", [64, 3 * C], BF16, isOutput=False)
    msk = nc.declare_dram_parameter("msk", [128, 4 * QT], BF16, isOutput=False)
    out = nc.declare_dram_parameter("out", [C, t], BF16, isOutput=True)

    xT_r = xT.ap().rearrange("(a p) t -> p a t", p=128)
    wqk_r = wqk.ap().rearrange("(a p) m -> p a m", p=128)
    wv_r = wv.ap().rearrange("(a p) m -> p a m", p=128)

    with tile.TileContext(nc) as tc:
        with (
            tc.tile_pool(name="const", bufs=1) as const_pool,
            tc.tile_pool(name="xt", bufs=3) as xp,
            tc.tile_pool(name="qp", bufs=nt) as qp,
            tc.tile_pool(name="kp", bufs=nt) as kpool,
            tc.tile_pool(name="q2p", bufs=nt) as q2p,
            tc.tile_pool(name="k2p", bufs=nt) as k2p,
            tc.tile_pool(name="vp", bufs=nsub) as vp,
            tc.tile_pool(name="yp", bufs=HPC + 1) as yp,
            tc.tile_pool(name="ep", bufs=8) as ep,
            tc.tile_pool(name="op", bufs=4) as op,
            tc.tile_pool(name="sp", bufs=6) as sp,
            tc.tile_pool(name="ps_qk", bufs=2, space="PSUM") as ps_qk,
            tc.tile_pool(name="ps_y", bufs=HPC, space="PSUM") as ps_y,
            tc.tile_pool(name="ps_pr", bufs=1, space="PSUM") as ps_pr,
        ):
            # ---- constants ----
            wqk_sb = const_pool.tile([128, NCH, 384], BF16, tag="wqk")
            nc.sync.dma_start(out=wqk_sb, in_=wqk_r)
            wv_sb = const_pool.tile([128, NCH, 192], BF16, tag="wv")
            nc.sync.dma_start(out=wv_sb, in_=wv_r)
            wo_sb = const_pool.tile([64, 3 * C], BF16, tag="wo")
            nc.sync.dma_start(out=wo_sb, in_=wo.ap())
            bqk_sb = const_pool.tile([128, 4], F32, tag="bqk")
            nc.sync.dma_start(out=bqk_sb, in_=bqk.ap())
            mask_sb = const_pool.tile([128, 4, QT], BF16, tag="msk")
            msk_r = msk.ap().rearrange("p (o q) -> p o q", q=QT)
            nc.sync.dma_start(out=mask_sb, in_=msk_r)

            q_t, k_t = [None] * nt, [None] * nt
            q2_t, k2_t = [None] * nt, [None] * nt   # [128,512], rows 64-127
            v_t = [None] * nsub
            x_t = [None] * nt
            y_t = [[None] * nt for _ in range(HPC)]

            def fetch_x(ti):
                xt = xp.tile([128, NCH, QT], BF16, tag="xt")
                nc.sync.dma_start(out=xt, in_=xT_r[:, :, ti * QT:(ti + 1) * QT])
                x_t[ti] = xt

            def emit_proj_piece(ti, piece):
                """pieces 0-3: Q01/K01/Q2/K2 m-tiles; 4-7: V k-subtiles."""
                xt = x_t[ti]
                if piece < 4:
                    c0, c1 = M_COLS[piece]
                    mp = c1 - c0
                    r0 = 0 if piece < 2 else 64   # head2 at rows 64-127
                    ps = ps_pr.tile([128, QT], F32, tag="pp")
                    for ci in range(NCH):
                        nc.tensor.matmul(
                            ps[r0:r0 + mp, :],
                            lhsT=wqk_sb[:, ci, c0:c1],
                            rhs=xt[:, ci, :],
                            start=(ci == 0),
                            stop=(ci == NCH - 1),
                        )
                    if piece < 2:
                        pool = (qp, kpool)[piece]
                        dst = pool.tile([128, QT], BF16, tag=("q", "k")[piece])
                        nc.vector.tensor_scalar_add(
                            dst, ps, bqk_sb[:, piece:piece + 1])
                        (q_t, k_t)[piece][ti] = dst
                    else:
                        pool = (q2p, k2p)[piece - 2]
                        full = pool.tile([128, QT], BF16, tag=("q2", "k2")[piece - 2])
                        nc.vector.tensor_scalar_add(
                            full[64:128, :], ps[64:128, :],
                            bqk_sb[64:128, piece:piece + 1])
                        (q2_t, k2_t)[piece - 2][ti] = full
                else:
                    si = piece - 4
                    ps = ps_pr.tile([128, QT], F32, tag="pp")
                    for ci in range(NCH):
                        nc.tensor.matmul(
                            ps[:, 0:HPC * 64],
                            lhsT=xt[:, ci, si * 128:(si + 1) * 128],
                            rhs=wv_sb[:, ci, :],
                            start=(ci == 0),
                            stop=(ci == NCH - 1),
                        )
                    vt = vp.tile([128, HPC * 65], BF16, tag="v")
                    vt_r = vt.rearrange("p (h e) -> p h e", e=65)
                    nc.vector.memset(vt_r[:, :, 64:65], 1.0)
                    nc.vector.tensor_copy(
                        vt_r[:, :, 0:64],
                        ps[:, 0:HPC * 64].rearrange("p (h e) -> p h e", e=64),
                    )
                    v_t[ti * 4 + si] = vt

            def normalize(h, qi, y_ps):
                # y_ps row 64 = denominator. DVE-copy to SBUF partition 64,
                # DMA to partition 0, broadcast on GpSimd, reciprocal, scale.
                den64 = sp.tile([65, QT], F32, tag="den64")
                nc.vector.tensor_copy(den64[64:65, :], y_ps[64:65, :])
                den = sp.tile([1, QT], F32, tag="den")
                nc.sync.dma_start(out=den, in_=den64[64:65, :])
                bc_sb = sp.tile([64, QT], F32, tag="bcs")
                nc.gpsimd.partition_broadcast(bc_sb, den[0:1, :])
                rec = sp.tile([64, QT], F32, tag="rec")
                nc.vector.reciprocal_approx_fast(rec, bc_sb)
                yt = yp.tile([64, QT], BF16, tag="y")
                nc.vector.tensor_mul(yt, y_ps[0:64, :], rec)
                y_t[h][qi] = yt

            def emit_outproj(qi):
                for mo in range(NCH):
                    ps = ps_pr.tile([128, QT], F32, tag="pp")
                    for h in range(HPC):
                        nc.tensor.matmul(
                            ps,
                            lhsT=wo_sb[:, h * C + mo * 128:h * C + (mo + 1) * 128],
                            rhs=y_t[h][qi],
                            start=(h == 0),
                            stop=(h == HPC - 1),
                        )
                    ot = op.tile([128, QT], BF16, tag="o")
                    nc.vector.tensor_copy(ot, ps)
                    nc.sync.dma_start(
                        out=out.ap()[mo * 128:(mo + 1) * 128,
                                     qi * QT:(qi + 1) * QT],
                        in_=ot,
                    )
                    yield

            # ---- pipeline ----
            fetch_x(0)
            for piece in range(8):
                emit_proj_piece(0, piece)

            for qi in range(nt):
                n_k = 4 * qi + 4
                if qi + 1 < nt:
                    fetch_x(qi + 1)
                pending = list(range(8)) if qi + 1 < nt else []
                # out-proj of the previous q-tile: interleave its pieces with
                # head2's QK (PE rows 64-127 vs 0-63 -> they overlap)
                oproj = emit_outproj(qi - 1) if qi > 0 else iter(())

                # y2 first: its buffer is the first freed (normalize(2) is
                # hoisted right after the h2 loop), so h2's attV of the next
                # q-tile never head-of-line blocks the PE stream.
                y2 = ps_y.tile([65, QT], F32, tag="psy")
                y0 = ps_y.tile([65, QT], F32, tag="psy")
                y1 = ps_y.tile([65, QT], F32, tag="psy")

                # head 2 (PE rows 64-127): pairs of k-tiles per Exp
                def qlo_of(kt):
                    o = kt - 4 * qi
                    return 128 * o if (o > 0 and qi > 0) else 0

                q2_ap = q2_t[qi][64:128, :]
                for pj in range(n_k // 2):
                    kts = (2 * pj, 2 * pj + 1)
                    qlo0 = qlo_of(kts[0])  # <= qlo_of(kts[1])
                    aps = ps_qk.tile([128, 2, QT], F32, tag="ps")
                    for u in (0, 1):
                        kt = kts[u]
                        tj, tcol = kt // 4, (kt % 4) * 128
                        # write from the pair's min trim so the shared Exp
                        # reads only matmul-written columns
                        nc.tensor.matmul(
                            aps[:, u, qlo0:], lhsT=k2_t[tj][64:128, tcol:tcol + 128],
                            rhs=q2_ap[:, qlo0:], start=True, stop=True,
                        )
                    et = ep.tile([128, 2, QT], BF16, tag="e")
                    nc.scalar.activation(et[:, :, qlo0:], aps[:, :, qlo0:], AF.Exp)
                    for u in (0, 1):
                        o = kts[u] - 4 * qi
                        if o >= 0:
                            qlo = qlo_of(kts[u])
                            nc.vector.tensor_mul(
                                et[:, u, qlo:], et[:, u, qlo:], mask_sb[:, o, qlo:]
                            )
                    for u in (0, 1):
                        kt = kts[u]
                        qlo = qlo_of(kt)
                        nc.tensor.matmul(
                            y2[:, qlo:],
                            lhsT=v_t[kt][:, 2 * 65:3 * 65],
                            rhs=et[:, u, qlo:],
                            start=(kt == 0),
                            stop=(kt == n_k - 1),
                        )
                    next(oproj, None)
                    next(oproj, None)
                for _ in oproj:
                    pass
                normalize(2, qi, y2)

                # heads 0/1 interleaved per k-tile: PE row groups 0-63/64-127
                q0_ap = q_t[qi][0:64, :]
                q1_ap = q_t[qi][64:128, :]
                for kt in range(n_k):
                    tj, tcol = kt // 4, (kt % 4) * 128
                    o = kt - 4 * qi
                    qlo = 128 * o if (o > 0 and qi > 0) else 0
                    aps = ps_qk.tile([128, 2, QT], F32, tag="ps")
                    nc.tensor.matmul(
                        aps[:, 0, qlo:], lhsT=k_t[tj][0:64, tcol:tcol + 128],
                        rhs=q0_ap[:, qlo:], start=True, stop=True,
                    )
                    nc.tensor.matmul(
                        aps[:, 1, qlo:], lhsT=k_t[tj][64:128, tcol:tcol + 128],
                        rhs=q1_ap[:, qlo:], start=True, stop=True,
                    )
                    et = ep.tile([128, 2, QT], BF16, tag="e")
                    nc.scalar.activation(et[:, :, qlo:], aps[:, :, qlo:], AF.Exp)
                    if o >= 0:
                        nc.vector.tensor_mul(
                            et[:, :, qlo:], et[:, :, qlo:],
                            mask_sb[:, o:o + 1, qlo:].to_broadcast(
                                [128, 2, QT - qlo]),
                        )
                    for u, yps in ((0, y0), (1, y1)):
                        nc.tensor.matmul(
                            yps[:, qlo:],
                            lhsT=v_t[kt][:, u * 65:(u + 1) * 65],
                            rhs=et[:, u, qlo:],
                            start=(kt == 0),
                            stop=(kt == n_k - 1),
                        )
                    if pending:
                        emit_proj_piece(qi + 1, pending.pop(0))
                while pending:
                    emit_proj_piece(qi + 1, pending.pop(0))

                normalize(0, qi, y0)
                normalize(1, qi, y1)

            for _ in emit_outproj(nt - 1):
                pass

    nc.compile()
    return nc


def make_mask():
    i = np.arange(128)[:, None]
    j = np.arange(QT)[None, :]
    m = np.zeros((128, 4 * QT), np.float32)
    for o in range(4):
        m[:, o * QT:(o + 1) * QT] = (j >= o * 128 + i)
    return m


def shard_inputs(x, Wq, bq, Wk, bk, Wv, bv, Wo, bo, t=T):
    """Build per-core in_maps."""
    s = 1.0 / np.sqrt(D)
    mask = make_mask()
    bf = ml_dtypes.bfloat16
    in_maps = []
    for c in range(N_CORES):
        b = c // (N_CORES // x.shape[0])
        h0 = HPC * (c % 4)
        hs = slice(h0 * D, (h0 + HPC) * D)
        Wq_s = (Wq[hs] * s).astype(np.float32)
        bq_s = (bq[hs] * s).astype(np.float32)
        Wk_s, bk_s = Wk[hs], bk[hs]
        wqk = np.concatenate(
            [Wq_s[0:128].T, Wk_s[0:128].T, Wq_s[128:192].T, Wk_s[128:192].T], axis=1
        )  # [768, 384]
        bqk = np.zeros((128, 4), np.float32)
        bqk[:, 0] = bq_s[0:128]
        bqk[:, 1] = bk_s[0:128]
        bqk[64:128, 2] = bq_s[128:192]
        bqk[64:128, 3] = bk_s[128:192]
        wv = np.ascontiguousarray(Wv[hs].T)
        wo = np.concatenate(
            [Wo[:, hs][:, h * D:(h + 1) * D].T for h in range(HPC)], axis=1
        )  # [64, 3*768]
        in_maps.append({
            "xT": np.ascontiguousarray(x[b].T).astype(bf),
            "wqk": np.ascontiguousarray(wqk).astype(bf),
            "bqk": np.ascontiguousarray(bqk),
            "wv": wv.astype(bf),
            "wo": np.ascontiguousarray(wo).astype(bf),
            "msk": mask.astype(bf),
        })
    return in_maps


_NC_CACHE = {}


def get_nc(t=T):
    if t not in _NC_CACHE:
        _NC_CACHE[t] = build_nc(t)
    return _NC_CACHE[t]


def run_cores(in_maps, t=T, trace=False, tmpdir=None):
    from concourse.bass_utils import run_bass_kernel_spmd

    nc = get_nc(t)
    return run_bass_kernel_spmd(
        nc, in_maps, list(range(N_CORES)), trace=trace, tmpdir=tmpdir
    )


def gather(results, x_shape, bv, Wo, bo):
    B, t, _ = x_shape
    out = np.zeros((B, t, C), np.float32)
    for c in range(N_CORES):
        b = c // (N_CORES // B)
        out[b] += results[c]["out"].T.astype(np.float32)
    out += (bv @ Wo.T + bo)[None, None, :]
    return out


def kernel(x, Wq, bq, Wk, bk, Wv, bv, Wo, bo, _trace=False, _tmpdir=None):
    x = np.asarray(x, dtype=np.float32)
    args = [np.asarray(a, dtype=np.float32) for a in (Wq, bq, Wk, bk, Wv, bv, Wo, bo)]
    Wq, bq, Wk, bk, Wv, bv, Wo, bo = args
    t = x.shape[1]
    in_maps = shard_inputs(x, Wq, bq, Wk, bk, Wv, bv, Wo, bo, t=t)
    res = run_cores(in_maps, t=t, trace=_trace, tmpdir=_tmpdir)
    out = gather(res.results, x.shape, bv, Wo, bo)
    kernel.last_result = res
    return out
